# revision 36
# baseline (speedup 1.0000x reference)
"""Causal attention with RoPE, tensor-parallel over 8 NeuronCores.

Problem: B=2, L=2048, d_model=2048, H=16 heads, D=128 head dim.
  qkv = X @ W_qkv  (per-head [q|k|v] column layout)
  Q,K rope'd (interleaved pairs), causal softmax(QK^T/sqrt(D)) @ V, @ W_out.

Sharding (Megatron-style): core c handles batch b=c//4 and head group
g=c%4 (4 heads). Each core computes a partial output
(attn-out of its heads) @ (its W_out rows); host sums the 4 partials per
batch.

Per-core dataflow (all matmuls in float32r = fp32 with 12-bit mantissa):
  Phase A: stream X^T tiles, project Q/K/V (natural [l,d] layout),
           RoPE Q,K on DVE, PE-transpose to Q^T/K^T [d,l]; V stays in
           SBUF, Q^T/K^T spill to DRAM (SBUF pressure).
  Phase B: per head: S^T[k,q] = K^T-block.T @ Q^T-chunk on PE, exp on
           ACT (scale 1/sqrt(D)), diagonal causal masks on DVE (blocks
           trimmed to the causal q-range), colsum via ones-matmul,
           O^T accum = V.T @ expS^T, normalize via DVE reciprocal +
           GPSIMD partition_broadcast.
  Phase C: Y = O^T.T @ W_out-rows, accumulated over heads, DMA out.
"""
import math
import numpy as np
import concourse.bacc as bacc
import concourse.mybir as mybir
import concourse.tile as tile
from concourse.bass_utils import run_bass_kernel_spmd
from concourse.masks import make_identity

F32 = mybir.dt.float32
F32R = mybir.dt.float32r
AF = mybir.ActivationFunctionType

N_HEADS = 16
D = 128
THETA = 10000.0
B_FULL, L_FULL, DM_FULL = 2, 2048, 2048
H_PER_CORE = 4
N_CORES = 8


def build_nc(L=L_FULL, DM=DM_FULL, H=H_PER_CORE):
    """One NeuronCore's program. OUT width == DM."""
    LT = L // 128          # l-tiles
    KT = DM // 128         # contract tiles for projections
    HD = H * D             # qkv width per core
    QC = L // 512          # q-chunks
    OC = DM // 512         # out-proj n-chunks
    ISQ = 1.0 / math.sqrt(D)

    nc = bacc.Bacc(None, target_bir_lowering=False)

    xt_d = nc.dram_tensor("xt", [DM, L], F32R, kind="ExternalInput")
    wq_d = nc.dram_tensor("wq", [DM, HD], F32R, kind="ExternalInput")
    wk_d = nc.dram_tensor("wk", [DM, HD], F32R, kind="ExternalInput")
    wv_d = nc.dram_tensor("wv", [DM, HD], F32R, kind="ExternalInput")
    wo_d = nc.dram_tensor("wo", [HD, DM], F32R, kind="ExternalInput")
    cos_d = nc.dram_tensor("cosb", [L, H * 64], F32, kind="ExternalInput")
    sin_d = nc.dram_tensor("sinb", [L, H * 64], F32, kind="ExternalInput")
    mask_d = nc.dram_tensor("masks", [4, 128, 512], mybir.dt.bfloat16,
                             kind="ExternalInput")
    y_d = nc.dram_tensor("y", [L, DM], F32, kind="ExternalOutput")

    # spill space for phase A -> B handoff (Q^T only; K^T/V stay in SBUF)
    qt_d = nc.dram_tensor("qt_sp", [H, 128, L], F32R)

    with tile.TileContext(nc) as tc:
        with tc.tile_pool(name="const", bufs=1) as constp:
            id32 = constp.tile([128, 128], F32)
            make_identity(nc, id32[:])
            idr = constp.tile([128, 128], F32R)
            nc.vector.tensor_copy(idr[:], id32[:])
            # V resident across phases A/B: [l-in-tile, l-tile, h*d]
            v4 = constp.tile([128, LT, HD], F32R)
            # K^T resident for all heads (skip DRAM round-trip)
            ktres = constp.tile([128, H, L], F32R)
            # masks + first-chunk Q^T pools live early for prefetch
            maskp = constp
            masks_t = constp.tile([128, 4, 512], mybir.dt.bfloat16)

            qtp_outer = tc.tile_pool(name="qtc", bufs=4)
            qtp = qtp_outer.__enter__()
            qs_pre = {}
            for h in range(H):
                qs_pre[h] = qtp.tile([128, 512], F32R, tag="qs",
                                     name=f"qspre{h}")

            # ---------------- Phase A ----------------
            with (
                tc.tile_pool(name="wqkv", bufs=1) as wp,
                tc.tile_pool(name="xts", bufs=2) as xp,
                tc.tile_pool(name="trig", bufs=2) as trp,
                tc.tile_pool(name="stage", bufs=2) as stp,
                tc.tile_pool(name="spill", bufs=3) as spp,
                tc.tile_pool(name="psA", bufs=2, space="PSUM") as psA,
                tc.tile_pool(name="psT", bufs=2, space="PSUM") as psT,
            ):
                wq_t = wp.tile([128, KT, HD], F32R, tag="wq")
                wk_t = wp.tile([128, KT, HD], F32R, tag="wk")
                wv_t = wp.tile([128, KT, HD], F32R, tag="wv")
                wq_r = wq_d.rearrange("(j p) n -> p j n", p=128)
                wk_r = wk_d.rearrange("(j p) n -> p j n", p=128)
                wv_r = wv_d.rearrange("(j p) n -> p j n", p=128)
                xt_r = xt_d.rearrange("(j p) l -> p j l", p=128)

                xt_first = xp.tile([128, KT, 128], F32R, tag="xt",
                                   name="xt_first")
                nc.sync.dma_start(xt_first[:], xt_r[:, :, 0:128])
                for j in range(KT):
                    nc.sync.dma_start(wq_t[:, j], wq_r[:, j])
                    nc.sync.dma_start(wk_t[:, j], wk_r[:, j])
                    nc.sync.dma_start(wv_t[:, j], wv_r[:, j])
                nc.sync.dma_start(masks_t[:],
                                  mask_d.rearrange("m p q -> p m q"))

                for i in range(LT):
                    if i == 0:
                        xt_t = xt_first
                    else:
                        xt_t = xp.tile([128, KT, 128], F32R, tag="xt")
                        nc.sync.dma_start(
                            xt_t[:], xt_r[:, :, 128 * i:128 * (i + 1)])
                    cos_t = trp.tile([128, H * 64], F32, tag="cos")
                    sin_t = trp.tile([128, H * 64], F32, tag="sin")
                    nc.sync.dma_start(cos_t[:], cos_d[128 * i:128 * (i + 1), :])
                    nc.sync.dma_start(sin_t[:], sin_d[128 * i:128 * (i + 1), :])

                    psq = psA.tile([128, HD], F32, tag="psq")
                    psk = psA.tile([128, HD], F32, tag="psk")
                    psv = psA.tile([128, HD], F32, tag="psv")
                    for j in range(KT):
                        st, sp = (j == 0), (j == KT - 1)
                        nc.tensor.matmul(psq[:], xt_t[:, j], wq_t[:, j],
                                         start=st, stop=sp)
                        nc.tensor.matmul(psk[:], xt_t[:, j], wk_t[:, j],
                                         start=st, stop=sp)
                        nc.tensor.matmul(psv[:], xt_t[:, j], wv_t[:, j],
                                         start=st, stop=sp)

                    # V: psum -> resident SBUF tile
                    nc.vector.tensor_copy(v4[:, i, :], psv[:])

                    # RoPE on Q,K (pairs along free dim)
                    qrot = stp.tile([128, HD], F32R, tag="qrot")
                    krot = stp.tile([128, HD], F32R, tag="krot")
                    for ps, rot in ((psq, qrot), (psk, krot)):
                        pe = ps[:].rearrange("p (h i two) -> p two (h i)",
                                             two=2, h=H)
                        ro = rot[:].rearrange("p (h i two) -> p two (h i)",
                                              two=2, h=H)
                        x1, x2 = pe[:, 0], pe[:, 1]
                        t1 = stp.tile([128, H * 64], F32, tag="t1", bufs=1)
                        t2 = stp.tile([128, H * 64], F32, tag="t2", bufs=1)
                        nc.vector.tensor_mul(t1[:], x1, cos_t[:])
                        nc.vector.tensor_mul(t2[:], x2, sin_t[:])
                        nc.vector.tensor_sub(ro[:, 0], t1[:], t2[:])
                        t3 = stp.tile([128, H * 64], F32, tag="t3", bufs=1)
                        t4 = stp.tile([128, H * 64], F32, tag="t4", bufs=1)
                        nc.vector.tensor_mul(t3[:], x1, sin_t[:])
                        nc.vector.tensor_mul(t4[:], x2, cos_t[:])
                        nc.vector.tensor_add(ro[:, 1], t3[:], t4[:])

                    # transpose rope'd Q,K per head; K stays in SBUF,
                    # Q spills to DRAM
                    for h in range(H):
                        for rot in (qrot, krot):
                            pst = psT.tile([128, 128], F32R, tag="pst")
                            nc.tensor.transpose(
                                pst[:], rot[:, 128 * h:128 * (h + 1)], idr[:]
                            )
                            if rot is krot:
                                nc.vector.tensor_copy(
                                    ktres[:, h, 128 * i:128 * (i + 1)],
                                    pst[:])
                                continue
                            tsb = spp.tile([128, 128], F32R, tag="tsb")
                            nc.vector.tensor_copy(tsb[:], pst[:])
                            nc.sync.dma_start(
                                qt_d[h, :, 128 * i:128 * (i + 1)], tsb[:]
                            )
                    if i == min(3, LT - 1):
                        for h in range(H):
                            nc.sync.dma_start(
                                qs_pre[h][:], qt_d[h, :, 0:512])

            # ---------------- Phase B + C (c-major, interleaved) ----------
            with (
                tc.tile_pool(name="otp", bufs=2) as otpool,
                tc.tile_pool(name="wo", bufs=1) as wop,
                tc.tile_pool(name="est", bufs=QC * 4 + 2) as ep,
                tc.tile_pool(name="bn", bufs=3) as bp,
                tc.tile_pool(name="yst", bufs=3) as yp,
                tc.tile_pool(name="psS", bufs=3, space="PSUM") as psS,
                tc.tile_pool(name="psO", bufs=3, space="PSUM") as psO,
                tc.tile_pool(name="psY", bufs=2, space="PSUM") as psY,
            ):
                wo_t = wop.tile([128, H, DM], F32R, tag="wo")
                wo_r = wo_d.rearrange("(h p) n -> p h n", p=128)
                for h in range(H):
                    nc.sync.dma_start(wo_t[:, h], wo_r[:, h])
                c_order = ([0] + list(range(QC - 1, 0, -1))) if QC > 1 else [0]
                c0 = c_order[0]

                for c in c_order:
                    nblk = min(4 * (c + 1), LT)
                    ot_c = otpool.tile([128, H, 512], F32R, tag="ot")
                    for h in range(H):
                        kt_t = ktres[:, h]
                        if c == c0 and h in qs_pre:
                            qs = qs_pre[h]
                        else:
                            qs = qtp.tile([128, 512], F32R, tag="qs")
                            nc.sync.dma_start(
                                qs[:], qt_d[h, :, 512 * c:512 * (c + 1)])
                        es = []
                        offs = []
                        for j in range(nblk):
                            m = j - 4 * c
                            qo = 128 * m if m > 0 else 0
                            pss = psS.tile([128, 512], F32, tag="pss")
                            nc.tensor.matmul(
                                pss[:, qo:],
                                kt_t[:, 128 * j:128 * (j + 1)],
                                qs[:, qo:],
                                start=True, stop=True,
                            )
                            e = ep.tile([128, 512], F32R, tag="e")
                            nc.scalar.activation(e[:, qo:], pss[:, qo:],
                                                 AF.Exp, scale=ISQ)
                            if m >= 0:
                                nc.vector.tensor_mul(
                                    e[:, qo:], e[:, qo:],
                                    masks_t[:, m, qo:])
                            es.append(e)
                            offs.append(qo)
                        # denominators: per-block partition-reduce on GPSIMD,
                        # chained accumulate (valid q-ranges only)
                        acc = bp.tile([1, 512], F32, tag="acc")
                        for j in range(nblk):
                            qo = offs[j]
                            if j == 0:
                                nc.gpsimd.tensor_reduce(
                                    acc[:], es[0][:],
                                    axis=mybir.AxisListType.C,
                                    op=mybir.AluOpType.add)
                                continue
                            red = bp.tile([1, 512], F32, tag="red",
                                          name=f"red{j}")
                            nc.gpsimd.tensor_reduce(
                                red[:, qo:], es[j][:, qo:],
                                axis=mybir.AxisListType.C,
                                op=mybir.AluOpType.add)
                            nc.gpsimd.tensor_add(
                                acc[:, qo:], acc[:, qo:], red[:, qo:])
                        pso = psO.tile([128, 512], F32, tag="pso")
                        for j in range(nblk):
                            qo = offs[j]
                            nc.tensor.matmul(
                                pso[:, qo:],
                                v4[:, j, 128 * h:128 * (h + 1)],
                                es[j][:, qo:],
                                start=(j == 0), stop=(j == nblk - 1))
                        inv = bp.tile([1, 512], F32, tag="inv")
                        nc.vector.reciprocal(inv[:], acc[:])
                        bc = bp.tile([128, 512], F32, tag="bc")
                        nc.gpsimd.partition_broadcast(bc[:], inv[:])
                        nc.vector.tensor_mul(ot_c[:, h, :], pso[:], bc[:])

                    # out-projection for this chunk's l-tiles
                    for i in range(4 * c, min(4 * (c + 1), LT)):
                        li = i - 4 * c
                        for o in range(OC):
                            psy = psY.tile([128, 512], F32, tag="psy")
                            for h in range(H):
                                nc.tensor.matmul(
                                    psy[:],
                                    ot_c[:, h, 128 * li:128 * (li + 1)],
                                    wo_t[:, h, 512 * o:512 * (o + 1)],
                                    start=(h == 0), stop=(h == H - 1),
                                )
                            ysb = yp.tile([128, 512], F32, tag="ysb")
                            nc.vector.tensor_copy(ysb[:], psy[:])
                            nc.sync.dma_start(
                                y_d[128 * i:128 * (i + 1),
                                    512 * o:512 * (o + 1)],
                                ysb[:],
                            )

            qtp_outer.__exit__(None, None, None)

    nc.compile()
    return nc


def make_inputs_for_core(X, W_qkv, W_out, core, L=L_FULL, DM=DM_FULL,
                         H=H_PER_CORE):
    """Host-side sharding: core -> (batch, head-group) inputs."""
    b = core // 4
    g = core % 4
    heads = range(g * H, (g + 1) * H)

    xt = np.ascontiguousarray(X[b].T).astype(np.float32)
    wq3 = W_qkv.reshape(DM, -1, 3 * D)
    wq = np.concatenate([wq3[:, h, 0:D] for h in heads], axis=1)
    wk = np.concatenate([wq3[:, h, D:2 * D] for h in heads], axis=1)
    wv = np.concatenate([wq3[:, h, 2 * D:3 * D] for h in heads], axis=1)
    wo = W_out[g * H * D:(g + 1) * H * D, :]

    inv_freq = 1.0 / (THETA ** (np.arange(0, D, 2, dtype=np.float32) / D))
    ang = np.arange(L, dtype=np.float32)[:, None] * inv_freq[None, :]
    cos1 = np.cos(ang).astype(np.float32)
    sin1 = np.sin(ang).astype(np.float32)
    cosb = np.tile(cos1, (1, H))
    sinb = np.tile(sin1, (1, H))

    import ml_dtypes
    masks = np.zeros((4, 128, 512), dtype=ml_dtypes.bfloat16)
    kk = np.arange(128)[:, None]
    qq = np.arange(512)[None, :]
    for m in range(4):
        masks[m] = (qq - kk - 128 * m >= 0).astype(ml_dtypes.bfloat16)

    return {
        "xt": np.ascontiguousarray(xt),
        "wq": np.ascontiguousarray(wq.astype(np.float32)),
        "wk": np.ascontiguousarray(wk.astype(np.float32)),
        "wv": np.ascontiguousarray(wv.astype(np.float32)),
        "wo": np.ascontiguousarray(wo.astype(np.float32)),
        "cosb": np.ascontiguousarray(cosb),
        "sinb": np.ascontiguousarray(sinb),
        "masks": masks,
    }


_NC_CACHE = {}


def get_nc():
    if "nc" not in _NC_CACHE:
        _NC_CACHE["nc"] = build_nc()
    return _NC_CACHE["nc"]


def kernel(X, W_qkv, W_out):
    X = np.asarray(X, dtype=np.float32)
    W_qkv = np.asarray(W_qkv, dtype=np.float32)
    W_out = np.asarray(W_out, dtype=np.float32)
    nc = get_nc()
    in_maps = [
        make_inputs_for_core(X, W_qkv, W_out, c) for c in range(N_CORES)
    ]
    res = run_bass_kernel_spmd(nc, in_maps, list(range(N_CORES)))
    out = np.zeros((B_FULL, L_FULL, DM_FULL), dtype=np.float32)
    for c in range(N_CORES):
        out[c // 4] += res.results[c]["y"]
    return out
